# revision 21
# baseline (speedup 1.0000x reference)
"""BERT-base encoder layer on 8 Trainium2 NeuronCores (Bass/Tile).

Sharding: data-parallel over batch. Full inputs [32, 512, 768] split into 8
shards of 4 batches (2048 tokens); every core runs the same NEFF on its shard
(SPMD, no collectives); host concatenates the outputs.

v2 design (vs baseline):
- Host passes x pre-transposed (feature-major) -> no on-chip x transposes.
- Attention computes scores TRANSPOSED (S^T = K @ Q^T, keys on partitions):
  * the key-side mask becomes a per-partition bias folded into the Exp
    activation (kills all rank-1 mask matmuls),
  * exp output IS the P^T layout the A@V matmul needs (kills all 768
    probability-transpose matmuls per core),
  * softmax denominator comes from an all-ones stationary matmul that also
    broadcasts the sum across partitions; 1/sum is applied during the
    PSUM->SBUF copy of the attention output.
- Superphase A = QKV + attention + O-proj (stores xa = x + attn out),
  superphase B = SelfOutput GEMM + LN1 + FFN + LN2. Emission is interleaved
  so the ~31us/batch Exp window on the Scalar engine overlaps the V/O-proj
  GEMMs, and LN/gelu latencies in B hide under the next batch's GEMMs.

All GEMMs run on the PE in bf16 with fp32 PSUM accumulation; softmax and
layernorm statistics run in fp32. 1/sqrt(dk) is folded into Wq on the host.
"""

import os
import numpy as np
import ml_dtypes

B, S, E, H, DK, FF = 32, 512, 768, 12, 64, 3072
NCORES = 8
BL = B // NCORES          # batches per core = 4
T = BL * S                # tokens per core = 2048
EPS = 1e-12
MASK_NEG = -87.0          # stays inside exp-table range; exp() == 0 in fp32

_CACHE = {}


def _bf(a):
    return np.ascontiguousarray(np.asarray(a, np.float32).astype(ml_dtypes.bfloat16))


def _build(flags):
    import concourse.bass as bass
    import concourse.bacc as bacc
    import concourse.mybir as mybir
    import concourse.tile as tile
    from contextlib import ExitStack

    (use_bq, use_bk, use_bv, use_bo, use_bso, use_bi, use_bout,
     use_g1, use_b1, use_g2, use_b2) = flags

    AF = mybir.ActivationFunctionType
    OP = mybir.AluOpType
    AX = mybir.AxisListType
    BF16 = mybir.dt.bfloat16
    F32 = mybir.dt.float32

    nc = bacc.Bacc("TRN2", target_bir_lowering=False)

    d_xt = nc.dram_tensor("xt", (E, T), BF16, kind="ExternalInput")
    d_wq = nc.dram_tensor("wq", (E, E), BF16, kind="ExternalInput")
    d_wk = nc.dram_tensor("wk", (E, E), BF16, kind="ExternalInput")
    d_wv = nc.dram_tensor("wv", (E, E), BF16, kind="ExternalInput")
    d_wo = nc.dram_tensor("wo", (E, E), BF16, kind="ExternalInput")
    d_wso = nc.dram_tensor("wso", (E, E), BF16, kind="ExternalInput")
    d_wi = nc.dram_tensor("wi", (E, FF), BF16, kind="ExternalInput")
    d_wout = nc.dram_tensor("wout", (FF, E), BF16, kind="ExternalInput")
    # mask bias per key token: [128, BL*4] fp32, column b*4+jc holds tokens
    # jc*128..jc*128+127 of batch b (0 keep / -87 masked)
    d_mb = nc.dram_tensor("mbias", (128, BL * 4), F32, kind="ExternalInput")
    d_id = nc.dram_tensor("ident", (128, 128), BF16, kind="ExternalInput")
    d_ones = nc.dram_tensor("onesrow", (1, 512), BF16, kind="ExternalInput")
    d_ones64 = nc.dram_tensor("ones64", (128, 64), BF16, kind="ExternalInput")
    # bias rows: 0=bq/8, 1=bk, 2=bv, 3=bo, 4=bso, 5=bout, 6=bi (full FF width)
    d_brow = nc.dram_tensor("brow", (7, FF), BF16, kind="ExternalInput")
    d_bic = nc.dram_tensor("bicol", (128, FF // 128), F32, kind="ExternalInput")
    # gamma1 | gamma2 | beta1 | beta2 ([128, 768] each, partition-broadcast)
    # + a trailing always-zero [128, 512] scratch region
    d_gb = nc.dram_tensor("gb", (128, 4 * E + 512), F32, kind="ExternalInput")
    d_out = nc.dram_tensor("out", (T, E), F32, kind="ExternalOutput")

    KT_E = E // 128    # 6
    NT_B = S // 128    # 4
    FT = FF // 128     # 24
    HP = H // 2        # 6

    need_gb = use_g1 or use_b1 or use_g2 or use_b2
    need_brow = use_bq or use_bk or use_bv or use_bo or use_bso or use_bout

    with ExitStack() as ctx:
        tc = ctx.enter_context(tile.TileContext(nc))

        c_pool = ctx.enter_context(tc.tile_pool(name="consts", bufs=1))
        # Wso hoisted to outer scope so its DMA overlaps superphase A
        wso_pool = ctx.enter_context(tc.tile_pool(name="wso", bufs=KT_E))
        # xa = x + attention output (feature-major), crosses the A->B boundary
        xa_pool = ctx.enter_context(tc.tile_pool(name="xa", bufs=BL * KT_E))

        XA = {}    # (b, kt) -> [128, S] bf16 tile

        # ============== superphase A: QKV, attention, O-proj ==============
        with ExitStack() as sa:
            wq_pool = sa.enter_context(tc.tile_pool(name="wq", bufs=KT_E))
            wk_pool = sa.enter_context(tc.tile_pool(name="wk", bufs=KT_E))
            wv_pool = sa.enter_context(tc.tile_pool(name="wv", bufs=KT_E))
            wo_pool = sa.enter_context(tc.tile_pool(name="wo", bufs=KT_E))
            xt_pool = sa.enter_context(tc.tile_pool(name="xt",
                                                    bufs=3 * KT_E + 1))
            qt_pool = sa.enter_context(tc.tile_pool(name="qt", bufs=KT_E + 2))
            kt_pool = sa.enter_context(tc.tile_pool(name="kt", bufs=KT_E + 2))
            v_pool = sa.enter_context(tc.tile_pool(name="v", bufs=NT_B + 2))
            pexp_pool = sa.enter_context(tc.tile_pool(name="pexp", bufs=44))
            rsb_pool = sa.enter_context(tc.tile_pool(name="rsb", bufs=3))
            att_pool = sa.enter_context(tc.tile_pool(name="att",
                                                     bufs=2 * KT_E + 2))

            p_mm = sa.enter_context(tc.tile_pool(name="p_mm", bufs=2,
                                                 space="PSUM"))
            p_sc = sa.enter_context(tc.tile_pool(name="p_sc", bufs=4,
                                                 space="PSUM"))
            p_acc = sa.enter_context(tc.tile_pool(name="p_acc", bufs=2,
                                                  space="PSUM"))

            def load_xt(b):
                xts = []
                for k in range(KT_E):
                    t = xt_pool.tile([128, S], BF16, name="xtt", tag="xt")
                    nc.gpsimd.dma_start(
                        t[:, :], d_xt[k * 128:(k + 1) * 128, b * S:(b + 1) * S])
                    xts.append(t)
                return xts

            XT_cur = load_xt(0)

            # spread the startup loads across queue engines so the first
            # QK GEMMs aren't gated on a single serial DMA-trigger queue
            def _load(pool, dram, rows, eng, name, width=E):
                t = pool.tile([128, width], BF16, name=name, tag=name)
                eng.dma_start(t[:, :], dram[rows * 128:(rows + 1) * 128, :])
                return t

            mb = c_pool.tile_from(d_mb[:, :], name="mb", )
            ones64 = c_pool.tile_from(d_ones64[:, :], name="ones64")
            WQ = [_load(wq_pool, d_wq, k, nc.gpsimd, "wqt") for k in range(KT_E)]
            WK = [_load(wk_pool, d_wk, k, nc.sync, "wkt") for k in range(KT_E)]
            WV = [_load(wv_pool, d_wv, k, nc.sync, "wvt") for k in range(KT_E)]
            WO = [_load(wo_pool, d_wo, k, nc.sync, "wot") for k in range(KT_E)]
            WSO = [_load(wso_pool, d_wso, k, nc.sync, "wsot")
                   for k in range(KT_E)]
            ident = c_pool.tile_from(d_id[:, :], name="ident")
            ones = c_pool.tile_from(d_ones[:, :], name="ones")
            brow = (c_pool.tile_from(d_brow[:, :], name="brow")
                    if need_brow else None)
            gb = c_pool.tile_from(d_gb[:, :], name="gb") if need_gb else None

            ATT_prev, XT_prev = None, None

            for b in range(BL):
                XT = XT_cur
                if b + 1 < BL:
                    XT_next = load_xt(b + 1)

                QT, KTt = [None] * KT_E, [None] * KT_E
                V = [None] * NT_B
                PEXP = {}
                ATT = [None] * KT_E

                def qk(et):
                    for Wt, dstl, pool, ub, brx, tg in (
                            (WQ, QT, qt_pool, use_bq, 0, "qt"),
                            (WK, KTt, kt_pool, use_bk, 1, "kt")):
                        ps = p_mm.tile([128, S], F32, name="qkps", tag="mm")
                        for k in range(KT_E):
                            nc.tensor.matmul(
                                ps[:, :], Wt[k][:, et * 128:(et + 1) * 128],
                                XT[k][:, :],
                                start=(k == 0), stop=(k == KT_E - 1 and not ub))
                        if ub:
                            nc.tensor.matmul(
                                ps[:, :],
                                brow[brx:brx + 1, et * 128:(et + 1) * 128],
                                ones[0:1, 0:S], start=False, stop=True)
                        dstl[et] = pool.tile([128, S], BF16, name="qkt",
                                             tag=tg)
                        nc.vector.tensor_copy(dstl[et][:, :], ps[:, :])

                def sc(hp, it0):
                    # scores^T for head pair hp, key chunks it0, it0+1:
                    # psum[k-chunk, q] = (KT[h] chunk).T @ QT[h]; exp w/ mask
                    for it in (it0, it0 + 1):
                        pss = []
                        for hh in range(2):
                            o = hh * 64
                            ps = p_sc.tile([128, S], F32, name="scps",
                                           tag="sc")
                            nc.tensor.matmul(
                                ps[:, :],
                                KTt[hp][o:o + 64, it * 128:(it + 1) * 128],
                                QT[hp][o:o + 64, :], start=True, stop=True)
                            pss.append(ps)
                        for hh in range(2):
                            pe = pexp_pool.tile([128, S], BF16, name="pexp",
                                                tag="pe")
                            nc.scalar.activation(
                                pe[:, :], pss[hh][:, :], AF.Exp,
                                bias=mb[:, b * 4 + it:b * 4 + it + 1])
                            PEXP[(2 * hp + hh, it)] = pe

                def vproj(tt):
                    vt = v_pool.tile([128, E], BF16, name="vt", tag="v")
                    for ec, n in ((0, 512), (512, 256)):
                        ps = p_mm.tile([128, 512], F32, name="vps", tag="mm")
                        for k in range(KT_E):
                            nc.tensor.matmul(
                                ps[:, :n], XT[k][:, tt * 128:(tt + 1) * 128],
                                WV[k][:, ec:ec + n],
                                start=(k == 0),
                                stop=(k == KT_E - 1 and not use_bv))
                        if use_bv:
                            nc.tensor.matmul(
                                ps[:, :n], ones[0:1, 0:128],
                                brow[2:3, ec:ec + n], start=False, stop=True)
                        nc.vector.tensor_copy(vt[:, ec:ec + n], ps[:, :n])
                    V[tt] = vt

                def oproj(et):
                    ps = p_mm.tile([128, S], F32, name="ops", tag="mm")
                    for k in range(KT_E):
                        nc.tensor.matmul(
                            ps[:, :], WO[k][:, et * 128:(et + 1) * 128],
                            ATT_prev[k][:, :],
                            start=(k == 0),
                            stop=(k == KT_E - 1 and not use_bo))
                    if use_bo:
                        nc.tensor.matmul(
                            ps[:, :], brow[3:4, et * 128:(et + 1) * 128],
                            ones[0:1, 0:S], start=False, stop=True)
                    xat = xa_pool.tile([128, S], BF16, name="xat", tag="xa")
                    nc.vector.scalar_tensor_tensor(
                        xat[:, :], ps[:, :], 1.0, XT_prev[et][:, :],
                        op0=OP.mult, op1=OP.add)
                    XA[(b - 1, et)] = xat

                def av(hp):
                    aps = p_acc.tile([128, S], F32, name="avps", tag="acc")
                    sps = p_acc.tile([128, S], F32, name="smps", tag="acc")
                    for it in range(NT_B):
                        for hh in range(2):
                            o = hh * 64
                            nc.tensor.matmul(
                                aps[o:o + 64, :],
                                V[it][:, hp * 128 + o:hp * 128 + o + 64],
                                PEXP[(2 * hp + hh, it)][:, :],
                                start=(it == 0), stop=(it == NT_B - 1),
                                tile_position=(0, o))
                        for hh in range(2):
                            o = hh * 64
                            nc.tensor.matmul(
                                sps[o:o + 64, :], ones64[:, :],
                                PEXP[(2 * hp + hh, it)][:, :],
                                start=(it == 0), stop=(it == NT_B - 1),
                                tile_position=(0, o))
                    rsb = rsb_pool.tile([128, S], F32, name="rsb", tag="rsb")
                    # ~18-bit reciprocal, ~5x faster than nc.vector.reciprocal;
                    # sums are in [~1, 600] so no denorm/inf edge cases
                    nc.vector.reciprocal_approx_fast(rsb[:, :], sps[:, :])
                    at = att_pool.tile([128, S], BF16, name="attt", tag="att")
                    nc.vector.scalar_tensor_tensor(
                        at[:, :], aps[:, :], 1.0, rsb[:, :],
                        op0=OP.mult, op1=OP.mult)
                    ATT[hp] = at

                nop = lambda: None
                O = [(lambda et=et: oproj(et)) if b > 0 else nop
                     for et in range(KT_E)]
                # interleaved emission: exp window overlaps V/O-proj GEMMs
                sched = [
                    lambda: qk(0), lambda: qk(1),
                    lambda: sc(0, 0), lambda: qk(2),
                    lambda: sc(0, 2), lambda: qk(3),
                    lambda: sc(1, 0), lambda: qk(4),
                    lambda: sc(1, 2), lambda: qk(5),
                    lambda: sc(2, 0), lambda: vproj(0),
                    lambda: sc(2, 2), lambda: vproj(1),
                    lambda: sc(3, 0), lambda: vproj(2),
                    lambda: sc(3, 2), lambda: vproj(3),
                    lambda: sc(4, 0), lambda: av(0),
                    lambda: sc(4, 2), O[0],
                    lambda: sc(5, 0), O[1],
                    lambda: sc(5, 2), O[2],
                    lambda: av(1), O[3],
                    lambda: av(2), O[4],
                    lambda: av(3), O[5],
                    lambda: av(4), lambda: av(5),
                ]
                for seg in sched:
                    seg()

                ATT_prev = ATT
                XT_prev = XT
                if b + 1 < BL:
                    XT_cur = XT_next

            # O-projection for the last batch
            for et in range(KT_E):
                ps = p_mm.tile([128, S], F32, name="ops", tag="mm")
                for k in range(KT_E):
                    nc.tensor.matmul(
                        ps[:, :], WO[k][:, et * 128:(et + 1) * 128],
                        ATT_prev[k][:, :],
                        start=(k == 0), stop=(k == KT_E - 1 and not use_bo))
                if use_bo:
                    nc.tensor.matmul(
                        ps[:, :], brow[3:4, et * 128:(et + 1) * 128],
                        ones[0:1, 0:S], start=False, stop=True)
                xat = xa_pool.tile([128, S], BF16, name="xat", tag="xa")
                nc.vector.scalar_tensor_tensor(
                    xat[:, :], ps[:, :], 1.0, XT_prev[et][:, :],
                    op0=OP.mult, op1=OP.add)
                XA[(BL - 1, et)] = xat

        # ========= superphase B: SelfOutput GEMM + LN1, FFN, LN2 =========
        with ExitStack() as sb:
            wi_pool = sb.enter_context(tc.tile_pool(name="wi", bufs=KT_E))
            wout_pool = sb.enter_context(tc.tile_pool(name="wout", bufs=FT))
            b_pool = sb.enter_context(tc.tile_pool(name="b_consts", bufs=1))
            h_pool = sb.enter_context(tc.tile_pool(name="h",
                                                   bufs=2 * NT_B + 2))
            ht_pool = sb.enter_context(tc.tile_pool(name="ht", bufs=2))
            fft_pool = sb.enter_context(tc.tile_pool(name="fft", bufs=FT + 2))
            sq_pool = sb.enter_context(tc.tile_pool(name="sq", bufs=2))
            rs_pool = sb.enter_context(tc.tile_pool(name="rsd", bufs=2))
            out_pool = sb.enter_context(tc.tile_pool(name="outp", bufs=2))
            t_pool = sb.enter_context(tc.tile_pool(name="sb_s", bufs=12))

            p_mm = sb.enter_context(tc.tile_pool(name="pb_mm", bufs=6,
                                                 space="PSUM"))
            p_tr = sb.enter_context(tc.tile_pool(name="pb_tr", bufs=2,
                                                 space="PSUM"))

            WI = [wi_pool.tile_from(d_wi[k * 128:(k + 1) * 128, :], name="wit")
                  for k in range(KT_E)]
            WOUT = [wout_pool.tile_from(d_wout[f * 128:(f + 1) * 128, :],
                                        name="woutt") for f in range(FT)]
            bic = b_pool.tile_from(d_bic[:, :], name="bic") if use_bi else None
            # trailing columns of d_gb are always zero-filled by the host
            zeros = b_pool.tile_from(d_gb[:, 4 * E:4 * E + 512], name="zeros")

            def layernorm(chunks, h_dst, gcol, use_g, use_bb, resid=None):
                """chunks: [(psum_ap, col0, n)]; h_dst: [128, E] out.
                resid: parallel list of sbuf APs added to psum first.

                Stages the psum chunks into SBUF with a fused
                copy(+residual)+rowsum so the PSUM banks free immediately,
                and runs the final normalize on the Scalar engine so the DVE
                queue stays short. eps=1e-12 is dropped (var >> eps here)."""
                rtile = rs_pool.tile([128, E], F32, name="rt", tag="rsd")
                s1 = t_pool.tile([128, 1], F32, name="s1", tag="s1")
                s1b = t_pool.tile([128, 1], F32, name="s1b", tag="s1b")
                for i, ((ps, c0, n), acc) in enumerate(zip(chunks, (s1, s1b))):
                    rx = resid[i] if resid is not None else zeros[:, :n]
                    nc.vector.scalar_tensor_tensor(
                        rtile[:, c0:c0 + n], ps, 1.0, rx,
                        op0=OP.mult, op1=OP.add)
                    nc.vector.reduce_sum(acc[:, :], rtile[:, c0:c0 + n],
                                         axis=AX.X)
                srcs = [(rtile[:, c0:c0 + n], c0, n) for (_, c0, n) in chunks]
                mu_n = t_pool.tile([128, 1], F32, name="mun", tag="mun")
                nc.vector.tensor_scalar(           # mu_n = -(s1 + s1b)/E
                    mu_n[:, :], s1[:, :], s1b[:, :], -1.0 / E,
                    op0=OP.add, op1=OP.mult)
                ss = t_pool.tile([128, 1], F32, name="ssa", tag="ssa", bufs=34)
                ssb = t_pool.tile([128, 1], F32, name="ssb", tag="ssb",
                                  bufs=34)
                for (src, c0, n), acc in zip(srcs, (ss, ssb)):
                    sq = sq_pool.tile([128, 512], BF16, name="sqt", tag="sq")
                    nc.scalar.activation(sq[:, :n], src, AF.Square,
                                         accum_out=acc[:, :])
                v1 = t_pool.tile([128, 1], F32, name="v1", tag="v1")
                nc.vector.tensor_scalar(           # (ss+ssb)/E
                    v1[:, :], ss[:, :], ssb[:, :], 1.0 / E,
                    op0=OP.add, op1=OP.mult)
                musq = t_pool.tile([128, 1], F32, name="musq", tag="musq")
                nc.vector.scalar_tensor_tensor(    # mu^2
                    musq[:, :], mu_n[:, :], 1.0, mu_n[:, :],
                    op0=OP.mult, op1=OP.mult)
                var = t_pool.tile([128, 1], F32, name="var", tag="var")
                nc.vector.scalar_tensor_tensor(    # var = v1 - mu^2
                    var[:, :], musq[:, :], -1.0, v1[:, :],
                    op0=OP.mult, op1=OP.add)
                sd = t_pool.tile([128, 1], F32, name="sd", tag="sd")
                nc.scalar.sqrt(sd[:, :], var[:, :])
                rstd = t_pool.tile([128, 1], F32, name="rstd", tag="rstd")
                nc.vector.reciprocal(rstd[:, :], sd[:, :])
                mnr = t_pool.tile([128, 1], F32, name="mnr", tag="mnr")
                nc.vector.scalar_tensor_tensor(    # -mu * rstd
                    mnr[:, :], mu_n[:, :], 1.0, rstd[:, :],
                    op0=OP.mult, op1=OP.mult)
                for (src, c0, n) in srcs:          # (x - mu) * rstd
                    nc.vector.tensor_scalar(
                        h_dst[:, c0:c0 + n], src, mu_n[:, :], rstd[:, :],
                        op0=OP.add, op1=OP.mult)
                if use_g:
                    nc.vector.scalar_tensor_tensor(
                        h_dst[:, :], h_dst[:, :], 1.0,
                        gb[:, gcol * E:(gcol + 1) * E],
                        op0=OP.mult, op1=OP.mult)
                if use_bb:
                    nc.vector.scalar_tensor_tensor(
                        h_dst[:, :], h_dst[:, :], 1.0,
                        gb[:, (gcol + 2) * E:(gcol + 3) * E],
                        op0=OP.mult, op1=OP.add)

            def emit_wso(b):
                """SelfOutput GEMM + LN1 -> h tiles (token-major bf16)."""
                hh_t = [None] * NT_B
                for tt in range(NT_B):
                    ch = []
                    for ec, n in ((0, 512), (512, 256)):
                        ps = p_mm.tile([128, 512], F32, name="sops", tag="mm")
                        for k in range(KT_E):
                            nc.tensor.matmul(
                                ps[:, :n],
                                XA[(b, k)][:, tt * 128:(tt + 1) * 128],
                                WSO[k][:, ec:ec + n],
                                start=(k == 0),
                                stop=(k == KT_E - 1 and not use_bso))
                        if use_bso:
                            nc.tensor.matmul(
                                ps[:, :n], ones[0:1, 0:128],
                                brow[4:5, ec:ec + n], start=False, stop=True)
                        ch.append((ps[:, :n], ec, n))
                    hh_t[tt] = h_pool.tile([128, E], BF16, name="hht", tag="h")
                    layernorm(ch, hh_t[tt], 0, use_g1, use_b1)
                return hh_t

            def emit_htrans(hh_t):
                hT = ht_pool.tile([128, KT_E * S], BF16, name="htt", tag="ht")
                for tt in range(NT_B):
                    tps = [p_tr.tile([128, 512], BF16, name="htp", tag="tr")
                           for _ in range(2)]
                    for et in range(KT_E):
                        sl = tps[et // 4][:, (et % 4) * 128:(et % 4 + 1) * 128]
                        nc.tensor.transpose(
                            sl, hh_t[tt][:, et * 128:(et + 1) * 128],
                            ident[:, :])
                    for et in range(KT_E):
                        sl = tps[et // 4][:, (et % 4) * 128:(et % 4 + 1) * 128]
                        nc.vector.tensor_copy(
                            hT[:, et * S + tt * 128:et * S + (tt + 1) * 128],
                            sl)
                return hT

            def emit_wi(hT):
                ffT = [None] * FT
                for ft in range(FT):
                    ps = p_mm.tile([128, 512], F32, name="fips", tag="mm")
                    for k in range(KT_E):
                        nc.tensor.matmul(
                            ps[:, :], WI[k][:, ft * 128:(ft + 1) * 128],
                            hT[:, k * S:k * S + 512],
                            start=(k == 0), stop=(k == KT_E - 1))
                    ffT[ft] = fft_pool.tile([128, 512], BF16, name="fftt",
                                            tag="fft")
                    if use_bi:
                        nc.scalar.activation(ffT[ft][:, :], ps[:, :], AF.Gelu,
                                             bias=bic[:, ft:ft + 1])
                    else:
                        nc.scalar.activation(ffT[ft][:, :], ps[:, :], AF.Gelu)
                return ffT

            def emit_wout(b, ffT, hh_t):
                t0 = b * S
                for tt in range(NT_B):
                    ch = []
                    for ec, n in ((0, 512), (512, 256)):
                        ps = p_mm.tile([128, 512], F32, name="wops", tag="mm")
                        for f in range(FT):
                            nc.tensor.matmul(
                                ps[:, :n],
                                ffT[f][:, tt * 128:(tt + 1) * 128],
                                WOUT[f][:, ec:ec + n],
                                start=(f == 0),
                                stop=(f == FT - 1 and not use_bout))
                        if use_bout:
                            nc.tensor.matmul(
                                ps[:, :n], ones[0:1, 0:128],
                                brow[5:6, ec:ec + n], start=False, stop=True)
                        ch.append((ps[:, :n], ec, n))
                    otile = out_pool.tile([128, E], F32, name="ot", tag="outp")
                    resid = [hh_t[tt][:, ec:ec + n] for (_, ec, n) in ch]
                    layernorm(ch, otile, 1, use_g2, use_b2, resid=resid)
                    nc.gpsimd.dma_start(
                        d_out[t0 + tt * 128:t0 + (tt + 1) * 128, :],
                        otile[:, :])

            # software pipeline: Wi(b-1)/Wout(b-1) hide LN1/gelu latencies;
            # two Wso batches up front cover the Wi/Wout weight DMA
            h_ = [None] * BL
            hT_ = [None] * BL
            ff_ = [None] * BL
            h_[0] = emit_wso(0)
            h_[1] = emit_wso(1)
            hT_[0] = emit_htrans(h_[0])
            for b in range(1, BL):
                ff_[b - 1] = emit_wi(hT_[b - 1])
                hT_[b] = emit_htrans(h_[b])
                emit_wout(b - 1, ff_[b - 1], h_[b - 1])
                if b + 1 < BL:
                    h_[b + 1] = emit_wso(b + 1)
            ff_[BL - 1] = emit_wi(hT_[BL - 1])
            emit_wout(BL - 1, ff_[BL - 1], h_[BL - 1])

    nc.compile()
    return nc


def _get_program(flags):
    key = ("prog", flags)
    if key not in _CACHE:
        _CACHE[key] = _build(flags)
    return _CACHE[key]


def kernel(x, mask, Wq, bq, Wk, bk, Wv, bv, Wo, bo,
           Wso, bso, gso, beso, Wi, bi, Wout, bout, gout, beout):
    from concourse.bass_utils import run_bass_kernel_spmd

    x = np.asarray(x, np.float32)
    mask = np.asarray(mask)
    sc = 1.0 / float(np.sqrt(np.float32(DK)))

    z = lambda a: not np.any(np.asarray(a))
    one = lambda a: bool(np.all(np.asarray(a) == 1.0))
    flags = (not z(bq), not z(bk), not z(bv), not z(bo), not z(bso),
             not z(bi), not z(bout),
             not one(gso), not z(beso), not one(gout), not z(beout))
    nc = _get_program(flags)

    wq_b = _bf(np.asarray(Wq, np.float32) * sc)
    wk_b, wv_b, wo_b, wso_b = _bf(Wk), _bf(Wv), _bf(Wo), _bf(Wso)
    wi_b, wout_b = _bf(Wi), _bf(Wout)
    identb = _bf(np.eye(128))
    onesr = _bf(np.ones((1, 512)))
    ones64 = _bf(np.ones((128, 64)))

    brow = np.zeros((7, FF), np.float32)
    brow[0, :E] = np.asarray(bq, np.float32) * sc
    for i, v in enumerate((bk, bv, bo, bso, bout)):
        brow[i + 1, :E] = v
    brow[6, :] = bi
    brow = _bf(brow)
    bicol = np.asarray(bi, np.float32).reshape(FF // 128, 128).T.copy()
    gbt = np.zeros((128, 4 * E + 512), np.float32)
    for i, g in enumerate((gso, gout, beso, beout)):   # g1|g2|b1|b2
        gbt[:, i * E:(i + 1) * E] = np.broadcast_to(
            np.asarray(g, np.float32).reshape(1, E), (128, E))

    in_maps = []
    for c in range(NCORES):
        xs = x[c * BL:(c + 1) * BL].reshape(T, E)
        xt = _bf(np.ascontiguousarray(xs.T))
        ms = np.asarray(mask[c * BL:(c + 1) * BL]).reshape(BL, S)
        # mbias[p, b*4 + jc] = 0/-87 for key token jc*128+p of batch b
        mbias = np.where(ms == 0, np.float32(MASK_NEG), np.float32(0.0))
        mbias = np.ascontiguousarray(
            mbias.reshape(BL, 4, 128).transpose(2, 0, 1).reshape(128, BL * 4)
        ).astype(np.float32)
        in_maps.append({
            "xt": xt, "wq": wq_b, "wk": wk_b, "wv": wv_b, "wo": wo_b,
            "wso": wso_b, "wi": wi_b, "wout": wout_b, "mbias": mbias,
            "ident": identb, "onesrow": onesr, "ones64": ones64,
            "brow": brow, "bicol": bicol, "gb": gbt,
        })

    trace = os.environ.get("KERNEL_TRACE", "0") == "1"
    res = run_bass_kernel_spmd(nc, in_maps, core_ids=list(range(NCORES)),
                               trace=trace)
    if trace and res.exec_time_ns is not None:
        print(f"HW exec time: {res.exec_time_ns} ns")
        if res.instructions_and_trace is not None:
            print(f"trace: {res.instructions_and_trace[1]}")
    out = np.concatenate([r["out"].reshape(BL, S, E) for r in res.results],
                         axis=0)
    return np.ascontiguousarray(out.astype(np.float32))


# revision 32
# speedup vs baseline: 1.0301x; 1.0301x over previous
"""BERT-base encoder layer on 8 Trainium2 NeuronCores (Bass/Tile).

Sharding: data-parallel over batch. Full inputs [32, 512, 768] split into 8
shards of 4 batches (2048 tokens); every core runs the same NEFF on its shard
(SPMD, no collectives); host concatenates the outputs.

v2 design (vs baseline):
- Host passes x pre-transposed (feature-major) -> no on-chip x transposes.
- Attention computes scores TRANSPOSED (S^T = K @ Q^T, keys on partitions):
  * the key-side mask becomes a per-partition bias folded into the Exp
    activation (kills all rank-1 mask matmuls),
  * exp output IS the P^T layout the A@V matmul needs (kills all 768
    probability-transpose matmuls per core),
  * softmax denominator comes from an all-ones stationary matmul that also
    broadcasts the sum across partitions; 1/sum is applied during the
    PSUM->SBUF copy of the attention output.
- Superphase A = QKV + attention + O-proj (stores xa = x + attn out),
  superphase B = SelfOutput GEMM + LN1 + FFN + LN2. Emission is interleaved
  so the ~31us/batch Exp window on the Scalar engine overlaps the V/O-proj
  GEMMs, and LN/gelu latencies in B hide under the next batch's GEMMs.

All GEMMs run on the PE in bf16 with fp32 PSUM accumulation; softmax and
layernorm statistics run in fp32. 1/sqrt(dk) is folded into Wq on the host.
"""

import os
import numpy as np
import ml_dtypes

B, S, E, H, DK, FF = 32, 512, 768, 12, 64, 3072
NCORES = 8
BL = B // NCORES          # batches per core = 4
T = BL * S                # tokens per core = 2048
EPS = 1e-12
MASK_NEG = -87.0          # stays inside exp-table range; exp() == 0 in fp32

_CACHE = {}


def _bf(a):
    return np.ascontiguousarray(np.asarray(a, np.float32).astype(ml_dtypes.bfloat16))


def _build(flags):
    import concourse.bass as bass
    import concourse.bacc as bacc
    import concourse.mybir as mybir
    import concourse.tile as tile
    from contextlib import ExitStack

    (use_bq, use_bk, use_bv, use_bo, use_bso, use_bi, use_bout,
     use_g1, use_b1, use_g2, use_b2) = flags

    AF = mybir.ActivationFunctionType
    OP = mybir.AluOpType
    AX = mybir.AxisListType
    BF16 = mybir.dt.bfloat16
    F32 = mybir.dt.float32

    nc = bacc.Bacc("TRN2", target_bir_lowering=False)

    d_xt = nc.dram_tensor("xt", (E, T), BF16, kind="ExternalInput")
    d_wq = nc.dram_tensor("wq", (E, E), BF16, kind="ExternalInput")
    d_wk = nc.dram_tensor("wk", (E, E), BF16, kind="ExternalInput")
    d_wv = nc.dram_tensor("wv", (E, E), BF16, kind="ExternalInput")
    d_wo = nc.dram_tensor("wo", (E, E), BF16, kind="ExternalInput")
    d_wso = nc.dram_tensor("wso", (E, E), BF16, kind="ExternalInput")
    d_wi = nc.dram_tensor("wi", (E, FF), BF16, kind="ExternalInput")
    d_wout = nc.dram_tensor("wout", (FF, E), BF16, kind="ExternalInput")
    # mask bias per key token: [128, BL*4] fp32, column b*4+jc holds tokens
    # jc*128..jc*128+127 of batch b (0 keep / -87 masked)
    d_mb = nc.dram_tensor("mbias", (128, BL * 4), F32, kind="ExternalInput")
    d_id = nc.dram_tensor("ident", (128, 128), BF16, kind="ExternalInput")
    d_ones = nc.dram_tensor("onesrow", (1, 512), BF16, kind="ExternalInput")
    d_ones64 = nc.dram_tensor("ones64", (128, 64), BF16, kind="ExternalInput")
    # bias rows: 0=bq/8, 1=bk, 2=bv, 3=bo, 4=bso, 5=bout, 6=bi (full FF width)
    d_brow = nc.dram_tensor("brow", (7, FF), BF16, kind="ExternalInput")
    d_bic = nc.dram_tensor("bicol", (128, FF // 128), F32, kind="ExternalInput")
    # gamma1 | gamma2 | beta1 | beta2 ([128, 768] each, partition-broadcast)
    # + a trailing always-zero [128, 512] scratch region
    d_gb = nc.dram_tensor("gb", (128, 4 * E + 512), F32, kind="ExternalInput")
    d_out = nc.dram_tensor("out", (T, E), F32, kind="ExternalOutput")

    KT_E = E // 128    # 6
    NT_B = S // 128    # 4
    FT = FF // 128     # 24
    HP = H // 2        # 6

    need_gb = use_g1 or use_b1 or use_g2 or use_b2
    need_brow = use_bq or use_bk or use_bv or use_bo or use_bso or use_bout

    with ExitStack() as ctx:
        tc = ctx.enter_context(tile.TileContext(nc))

        c_pool = ctx.enter_context(tc.tile_pool(name="consts", bufs=1))
        # Wso hoisted to outer scope so its DMA overlaps superphase A
        wso_pool = ctx.enter_context(tc.tile_pool(name="wso", bufs=KT_E))
        # xa = x + attention output (feature-major), crosses the A->B boundary
        xa_pool = ctx.enter_context(tc.tile_pool(name="xa", bufs=BL * KT_E))
        # LN machinery is shared between the phases: wso(0)+LN1(0) runs in
        # superphase A's tail (PE slack under the last exp window)
        h_pool = ctx.enter_context(tc.tile_pool(name="h", bufs=2 * NT_B + 2))
        sq_pool = ctx.enter_context(tc.tile_pool(name="sq", bufs=2))
        rs_pool = ctx.enter_context(tc.tile_pool(name="rsd", bufs=2))
        t_pool = ctx.enter_context(tc.tile_pool(name="sb_s", bufs=12))

        XA = {}    # (b, kt) -> [128, S] bf16 tile

        def layernorm(chunks, h_dst, gcol, use_g, use_bb, resid=None):
            """chunks: [(psum_ap, col0, n)]; h_dst: [128, E] out.
            resid: parallel list of sbuf APs added to psum first.

            Stages the psum chunks into SBUF immediately (frees the PSUM
            banks for the next GEMM group); eps=1e-12 dropped (var >> eps).
            References ones/gb/zeros, which are loaded before first use."""
            rtile = rs_pool.tile([128, E], F32, name="rt", tag="rsd")
            s1 = t_pool.tile([128, 1], F32, name="s1", tag="s1")
            s1b = t_pool.tile([128, 1], F32, name="s1b", tag="s1b")
            for i, ((ps, c0, n), acc) in enumerate(zip(chunks, (s1, s1b))):
                rx = resid[i] if resid is not None else zeros[:, :n]
                nc.vector.scalar_tensor_tensor(
                    rtile[:, c0:c0 + n], ps, 1.0, rx,
                    op0=OP.mult, op1=OP.add)
                nc.vector.reduce_sum(acc[:, :], rtile[:, c0:c0 + n],
                                     axis=AX.X)
            srcs = [(rtile[:, c0:c0 + n], c0, n) for (_, c0, n) in chunks]
            mu_n = t_pool.tile([128, 1], F32, name="mun", tag="mun")
            nc.vector.tensor_scalar(           # mu_n = -(s1 + s1b)/E
                mu_n[:, :], s1[:, :], s1b[:, :], -1.0 / E,
                op0=OP.add, op1=OP.mult)
            ss = t_pool.tile([128, 1], F32, name="ssa", tag="ssa", bufs=34)
            ssb = t_pool.tile([128, 1], F32, name="ssb", tag="ssb", bufs=34)
            for (src, c0, n), acc in zip(srcs, (ss, ssb)):
                sq = sq_pool.tile([128, 512], BF16, name="sqt", tag="sq")
                nc.scalar.activation(sq[:, :n], src, AF.Square,
                                     accum_out=acc[:, :])
            v1 = t_pool.tile([128, 1], F32, name="v1", tag="v1")
            nc.vector.tensor_scalar(           # (ss+ssb)/E
                v1[:, :], ss[:, :], ssb[:, :], 1.0 / E,
                op0=OP.add, op1=OP.mult)
            musq = t_pool.tile([128, 1], F32, name="musq", tag="musq")
            nc.vector.scalar_tensor_tensor(    # mu^2
                musq[:, :], mu_n[:, :], 1.0, mu_n[:, :],
                op0=OP.mult, op1=OP.mult)
            var = t_pool.tile([128, 1], F32, name="var", tag="var")
            nc.vector.scalar_tensor_tensor(    # var = v1 - mu^2
                var[:, :], musq[:, :], -1.0, v1[:, :],
                op0=OP.mult, op1=OP.add)
            sd = t_pool.tile([128, 1], F32, name="sd", tag="sd")
            nc.scalar.sqrt(sd[:, :], var[:, :])
            rstd = t_pool.tile([128, 1], F32, name="rstd", tag="rstd")
            nc.vector.reciprocal(rstd[:, :], sd[:, :])
            for (src, c0, n) in srcs:          # (x - mu) * rstd
                nc.vector.tensor_scalar(
                    h_dst[:, c0:c0 + n], src, mu_n[:, :], rstd[:, :],
                    op0=OP.add, op1=OP.mult)
            if use_g:
                nc.vector.scalar_tensor_tensor(
                    h_dst[:, :], h_dst[:, :], 1.0,
                    gb[:, gcol * E:(gcol + 1) * E],
                    op0=OP.mult, op1=OP.mult)
            if use_bb:
                nc.vector.scalar_tensor_tensor(
                    h_dst[:, :], h_dst[:, :], 1.0,
                    gb[:, (gcol + 2) * E:(gcol + 3) * E],
                    op0=OP.mult, op1=OP.add)

        def wso_tt(b, tt, pmm):
            """One token-tile of the SelfOutput GEMM + LN1 -> h tile."""
            ch = []
            for ec, n in ((0, 512), (512, 256)):
                ps = pmm.tile([128, 512], F32, name="sops", tag="mm")
                for k in range(KT_E):
                    nc.tensor.matmul(
                        ps[:, :n], XA[(b, k)][:, tt * 128:(tt + 1) * 128],
                        WSO[k][:, ec:ec + n],
                        start=(k == 0), stop=(k == KT_E - 1 and not use_bso))
                if use_bso:
                    nc.tensor.matmul(
                        ps[:, :n], ones[0:1, 0:128],
                        brow[4:5, ec:ec + n], start=False, stop=True)
                ch.append((ps[:, :n], ec, n))
            ht = h_pool.tile([128, E], BF16, name="hht", tag="h")
            layernorm(ch, ht, 0, use_g1, use_b1)
            return ht

        # ============== superphase A: QKV, attention, O-proj ==============
        with ExitStack() as sa:
            wq_pool = sa.enter_context(tc.tile_pool(name="wq", bufs=KT_E))
            wk_pool = sa.enter_context(tc.tile_pool(name="wk", bufs=KT_E))
            wv_pool = sa.enter_context(tc.tile_pool(name="wv", bufs=KT_E))
            wo_pool = sa.enter_context(tc.tile_pool(name="wo", bufs=KT_E))
            xt_pool = sa.enter_context(tc.tile_pool(name="xt",
                                                    bufs=3 * KT_E + 1))
            qt_pool = sa.enter_context(tc.tile_pool(name="qt", bufs=KT_E + 2))
            kt_pool = sa.enter_context(tc.tile_pool(name="kt", bufs=KT_E + 2))
            v_pool = sa.enter_context(tc.tile_pool(name="v", bufs=NT_B + 2))
            pexp_pool = sa.enter_context(tc.tile_pool(name="pexp", bufs=40))
            rsb_pool = sa.enter_context(tc.tile_pool(name="rsb", bufs=3))
            att_pool = sa.enter_context(tc.tile_pool(name="att",
                                                     bufs=2 * KT_E + 2))

            p_mm = sa.enter_context(tc.tile_pool(name="p_mm", bufs=2,
                                                 space="PSUM"))
            p_sc = sa.enter_context(tc.tile_pool(name="p_sc", bufs=4,
                                                 space="PSUM"))
            p_acc = sa.enter_context(tc.tile_pool(name="p_acc", bufs=2,
                                                  space="PSUM"))

            def load_xt(b):
                xts = []
                for k in range(KT_E):
                    t = xt_pool.tile([128, S], BF16, name="xtt", tag="xt")
                    nc.gpsimd.dma_start(
                        t[:, :], d_xt[k * 128:(k + 1) * 128, b * S:(b + 1) * S])
                    xts.append(t)
                return xts

            XT_cur = load_xt(0)

            # spread the startup loads across queue engines so the first
            # QK GEMMs aren't gated on a single serial DMA-trigger queue
            def _load(pool, dram, rows, eng, name, width=E):
                t = pool.tile([128, width], BF16, name=name, tag=name)
                eng.dma_start(t[:, :], dram[rows * 128:(rows + 1) * 128, :])
                return t

            WQ = [_load(wq_pool, d_wq, k, nc.sync, "wqt") for k in range(KT_E)]
            WK = [_load(wk_pool, d_wk, k, nc.scalar, "wkt") for k in range(KT_E)]
            mb = c_pool.tile_from(d_mb[:, :], name="mb", )
            ones64 = c_pool.tile_from(d_ones64[:, :], name="ones64")
            WV = [_load(wv_pool, d_wv, k, nc.sync, "wvt") for k in range(KT_E)]
            WO = [_load(wo_pool, d_wo, k, nc.sync, "wot") for k in range(KT_E)]
            WSO = [_load(wso_pool, d_wso, k, nc.sync, "wsot")
                   for k in range(KT_E)]
            ident = c_pool.tile_from(d_id[:, :], name="ident")
            ones = c_pool.tile_from(d_ones[:, :], name="ones")
            # trailing columns of d_gb are always zero-filled by the host
            zeros = c_pool.tile_from(d_gb[:, 4 * E:4 * E + 512], name="zeros")
            brow = (c_pool.tile_from(d_brow[:, :], name="brow")
                    if need_brow else None)
            gb = c_pool.tile_from(d_gb[:, :], name="gb") if need_gb else None

            ATT_prev, XT_prev = None, None
            H0 = []    # h(0) tiles produced in this phase's tail

            for b in range(BL):
                XT = XT_cur
                if b + 1 < BL:
                    XT_next = load_xt(b + 1)

                QT, KTt = [None] * KT_E, [None] * KT_E
                V = [None] * NT_B
                PEXP = {}
                ATT = [None] * KT_E

                def qk(et):
                    for Wt, dstl, pool, ub, brx, tg in (
                            (WQ, QT, qt_pool, use_bq, 0, "qt"),
                            (WK, KTt, kt_pool, use_bk, 1, "kt")):
                        ps = p_mm.tile([128, S], F32, name="qkps", tag="mm")
                        for k in range(KT_E):
                            nc.tensor.matmul(
                                ps[:, :], Wt[k][:, et * 128:(et + 1) * 128],
                                XT[k][:, :],
                                start=(k == 0), stop=(k == KT_E - 1 and not ub))
                        if ub:
                            nc.tensor.matmul(
                                ps[:, :],
                                brow[brx:brx + 1, et * 128:(et + 1) * 128],
                                ones[0:1, 0:S], start=False, stop=True)
                        dstl[et] = pool.tile([128, S], BF16, name="qkt",
                                             tag=tg)
                        nc.vector.tensor_copy(dstl[et][:, :], ps[:, :])

                def sc(hp, it0):
                    # scores^T for head pair hp, key chunks it0, it0+1:
                    # psum[k-chunk, q] = (KT[h] chunk).T @ QT[h]; exp w/ mask
                    for it in (it0, it0 + 1):
                        pss = []
                        for hh in range(2):
                            o = hh * 64
                            ps = p_sc.tile([128, S], F32, name="scps",
                                           tag="sc")
                            nc.tensor.matmul(
                                ps[:, :],
                                KTt[hp][o:o + 64, it * 128:(it + 1) * 128],
                                QT[hp][o:o + 64, :], start=True, stop=True)
                            pss.append(ps)
                        for hh in range(2):
                            pe = pexp_pool.tile([128, S], BF16, name="pexp",
                                                tag="pe")
                            nc.scalar.activation(
                                pe[:, :], pss[hh][:, :], AF.Exp,
                                bias=mb[:, b * 4 + it:b * 4 + it + 1])
                            PEXP[(2 * hp + hh, it)] = pe

                def vproj(tt):
                    vt = v_pool.tile([128, E], BF16, name="vt", tag="v")
                    for ec, n in ((0, 512), (512, 256)):
                        ps = p_mm.tile([128, 512], F32, name="vps", tag="mm")
                        for k in range(KT_E):
                            nc.tensor.matmul(
                                ps[:, :n], XT[k][:, tt * 128:(tt + 1) * 128],
                                WV[k][:, ec:ec + n],
                                start=(k == 0),
                                stop=(k == KT_E - 1 and not use_bv))
                        if use_bv:
                            nc.tensor.matmul(
                                ps[:, :n], ones[0:1, 0:128],
                                brow[2:3, ec:ec + n], start=False, stop=True)
                        nc.vector.tensor_copy(vt[:, ec:ec + n], ps[:, :n])
                    V[tt] = vt

                def oproj(et):
                    ps = p_mm.tile([128, S], F32, name="ops", tag="mm")
                    for k in range(KT_E):
                        nc.tensor.matmul(
                            ps[:, :], WO[k][:, et * 128:(et + 1) * 128],
                            ATT_prev[k][:, :],
                            start=(k == 0),
                            stop=(k == KT_E - 1 and not use_bo))
                    if use_bo:
                        nc.tensor.matmul(
                            ps[:, :], brow[3:4, et * 128:(et + 1) * 128],
                            ones[0:1, 0:S], start=False, stop=True)
                    xat = xa_pool.tile([128, S], BF16, name="xat", tag="xa")
                    nc.vector.scalar_tensor_tensor(
                        xat[:, :], ps[:, :], 1.0, XT_prev[et][:, :],
                        op0=OP.mult, op1=OP.add)
                    XA[(b - 1, et)] = xat

                def av(hp):
                    aps = p_acc.tile([128, S], F32, name="avps", tag="acc")
                    sps = p_acc.tile([128, S], F32, name="smps", tag="acc")
                    for it in range(NT_B):
                        for hh in range(2):
                            o = hh * 64
                            nc.tensor.matmul(
                                aps[o:o + 64, :],
                                V[it][:, hp * 128 + o:hp * 128 + o + 64],
                                PEXP[(2 * hp + hh, it)][:, :],
                                start=(it == 0), stop=(it == NT_B - 1),
                                tile_position=(0, o))
                        for hh in range(2):
                            o = hh * 64
                            nc.tensor.matmul(
                                sps[o:o + 64, :], ones64[:, :],
                                PEXP[(2 * hp + hh, it)][:, :],
                                start=(it == 0), stop=(it == NT_B - 1),
                                tile_position=(0, o))
                    rsb = rsb_pool.tile([128, S], F32, name="rsb", tag="rsb")
                    # ~18-bit reciprocal, ~5x faster than nc.vector.reciprocal;
                    # sums are in [~1, 600] so no denorm/inf edge cases
                    nc.vector.reciprocal_approx_fast(rsb[:, :], sps[:, :])
                    at = att_pool.tile([128, S], BF16, name="attt", tag="att")
                    nc.vector.scalar_tensor_tensor(
                        at[:, :], aps[:, :], 1.0, rsb[:, :],
                        op0=OP.mult, op1=OP.mult)
                    ATT[hp] = at

                nop = lambda: None
                O = [(lambda et=et: oproj(et)) if b > 0 else nop
                     for et in range(KT_E)]
                # batch 3 also runs wso(0)+LN1(0) under its exp-window slack
                W0 = ([(lambda tt=tt: H0.append(wso_tt(0, tt, p_mm)))
                       for tt in range(NT_B)] if b == BL - 1 else [nop] * 4)
                # interleaved emission: exp window overlaps V/O-proj GEMMs
                sched = [
                    lambda: qk(0), lambda: qk(1),
                    lambda: sc(0, 0), lambda: qk(2),
                    lambda: sc(0, 2), lambda: qk(3),
                    lambda: sc(1, 0), lambda: qk(4),
                    lambda: sc(1, 2), lambda: qk(5),
                    lambda: sc(2, 0), lambda: vproj(0),
                    lambda: sc(2, 2), lambda: vproj(1),
                    lambda: sc(3, 0), lambda: vproj(2),
                    lambda: sc(3, 2), lambda: vproj(3),
                    lambda: sc(4, 0), lambda: av(0),
                    lambda: sc(4, 2), O[0],
                    lambda: sc(5, 0), O[1],
                    lambda: sc(5, 2), O[2],
                    lambda: av(1), O[3],
                    lambda: av(2), O[4],
                    lambda: av(3), O[5],
                    W0[0], lambda: av(4),
                    W0[1], W0[2],
                    lambda: av(5), W0[3],
                ]
                for seg in sched:
                    seg()

                ATT_prev = ATT
                XT_prev = XT
                if b + 1 < BL:
                    XT_cur = XT_next

            # O-projection for the last batch
            for et in range(KT_E):
                ps = p_mm.tile([128, S], F32, name="ops", tag="mm")
                for k in range(KT_E):
                    nc.tensor.matmul(
                        ps[:, :], WO[k][:, et * 128:(et + 1) * 128],
                        ATT_prev[k][:, :],
                        start=(k == 0), stop=(k == KT_E - 1 and not use_bo))
                if use_bo:
                    nc.tensor.matmul(
                        ps[:, :], brow[3:4, et * 128:(et + 1) * 128],
                        ones[0:1, 0:S], start=False, stop=True)
                xat = xa_pool.tile([128, S], BF16, name="xat", tag="xa")
                nc.vector.scalar_tensor_tensor(
                    xat[:, :], ps[:, :], 1.0, XT_prev[et][:, :],
                    op0=OP.mult, op1=OP.add)
                XA[(BL - 1, et)] = xat

        # ========= superphase B: SelfOutput GEMM + LN1, FFN, LN2 =========
        with ExitStack() as sb:
            wi_pool = sb.enter_context(tc.tile_pool(name="wi", bufs=KT_E))
            wout_pool = sb.enter_context(tc.tile_pool(name="wout", bufs=FT))
            b_pool = sb.enter_context(tc.tile_pool(name="b_consts", bufs=1))
            ht_pool = sb.enter_context(tc.tile_pool(name="ht", bufs=2))
            fft_pool = sb.enter_context(tc.tile_pool(name="fft", bufs=FT + 2))
            out_pool = sb.enter_context(tc.tile_pool(name="outp", bufs=2))

            p_mm = sb.enter_context(tc.tile_pool(name="pb_mm", bufs=6,
                                                 space="PSUM"))
            p_tr = sb.enter_context(tc.tile_pool(name="pb_tr", bufs=2,
                                                 space="PSUM"))

            WI = [wi_pool.tile_from(d_wi[k * 128:(k + 1) * 128, :], name="wit")
                  for k in range(KT_E)]
            WOUT = [wout_pool.tile_from(d_wout[f * 128:(f + 1) * 128, :],
                                        name="woutt") for f in range(FT)]
            bic = b_pool.tile_from(d_bic[:, :], name="bic") if use_bi else None

            def emit_htrans(hh_t):
                hT = ht_pool.tile([128, KT_E * S], BF16, name="htt", tag="ht")
                for tt in range(NT_B):
                    tps = [p_tr.tile([128, 512], BF16, name="htp", tag="tr")
                           for _ in range(2)]
                    for et in range(KT_E):
                        sl = tps[et // 4][:, (et % 4) * 128:(et % 4 + 1) * 128]
                        nc.tensor.transpose(
                            sl, hh_t[tt][:, et * 128:(et + 1) * 128],
                            ident[:, :])
                    for et in range(KT_E):
                        sl = tps[et // 4][:, (et % 4) * 128:(et % 4 + 1) * 128]
                        nc.vector.tensor_copy(
                            hT[:, et * S + tt * 128:et * S + (tt + 1) * 128],
                            sl)
                return hT

            def emit_wi(hT):
                ffT = [None] * FT
                for ft in range(FT):
                    ps = p_mm.tile([128, 512], F32, name="fips", tag="mm")
                    for k in range(KT_E):
                        nc.tensor.matmul(
                            ps[:, :], WI[k][:, ft * 128:(ft + 1) * 128],
                            hT[:, k * S:k * S + 512],
                            start=(k == 0), stop=(k == KT_E - 1))
                    ffT[ft] = fft_pool.tile([128, 512], BF16, name="fftt",
                                            tag="fft")
                    if use_bi:
                        nc.scalar.activation(ffT[ft][:, :], ps[:, :], AF.Gelu,
                                             bias=bic[:, ft:ft + 1])
                    else:
                        nc.scalar.activation(ffT[ft][:, :], ps[:, :], AF.Gelu)
                return ffT

            def emit_wout(b, ffT, hh_t):
                t0 = b * S
                for tt in range(NT_B):
                    ch = []
                    for ec, n in ((0, 512), (512, 256)):
                        ps = p_mm.tile([128, 512], F32, name="wops", tag="mm")
                        for f in range(FT):
                            nc.tensor.matmul(
                                ps[:, :n],
                                ffT[f][:, tt * 128:(tt + 1) * 128],
                                WOUT[f][:, ec:ec + n],
                                start=(f == 0),
                                stop=(f == FT - 1 and not use_bout))
                        if use_bout:
                            nc.tensor.matmul(
                                ps[:, :n], ones[0:1, 0:128],
                                brow[5:6, ec:ec + n], start=False, stop=True)
                        ch.append((ps[:, :n], ec, n))
                    otile = out_pool.tile([128, E], F32, name="ot", tag="outp")
                    resid = [hh_t[tt][:, ec:ec + n] for (_, ec, n) in ch]
                    layernorm(ch, otile, 1, use_g2, use_b2, resid=resid)
                    # sync queue keeps the slow gpsimd drain off the tail
                    nc.sync.dma_start(
                        d_out[t0 + tt * 128:t0 + (tt + 1) * 128, :],
                        otile[:, :])

            # software pipeline: Wi(b-1)/Wout(b-1) hide LN1/gelu latencies;
            # h(0) already computed in superphase A's tail (H0)
            h_ = [None] * BL
            hT_ = [None] * BL
            ff_ = [None] * BL
            h_[0] = H0
            hT_[0] = emit_htrans(h_[0])
            for b in range(1, BL):
                h_[b] = [wso_tt(b, tt, p_mm) for tt in range(NT_B)]
                ff_[b - 1] = emit_wi(hT_[b - 1])
                hT_[b] = emit_htrans(h_[b])
                emit_wout(b - 1, ff_[b - 1], h_[b - 1])
            ff_[BL - 1] = emit_wi(hT_[BL - 1])
            emit_wout(BL - 1, ff_[BL - 1], h_[BL - 1])

    nc.compile()
    return nc


def _get_program(flags):
    key = ("prog", flags)
    if key not in _CACHE:
        _CACHE[key] = _build(flags)
    return _CACHE[key]


def kernel(x, mask, Wq, bq, Wk, bk, Wv, bv, Wo, bo,
           Wso, bso, gso, beso, Wi, bi, Wout, bout, gout, beout):
    from concourse.bass_utils import run_bass_kernel_spmd

    x = np.asarray(x, np.float32)
    mask = np.asarray(mask)
    sc = 1.0 / float(np.sqrt(np.float32(DK)))

    z = lambda a: not np.any(np.asarray(a))
    one = lambda a: bool(np.all(np.asarray(a) == 1.0))
    flags = (not z(bq), not z(bk), not z(bv), not z(bo), not z(bso),
             not z(bi), not z(bout),
             not one(gso), not z(beso), not one(gout), not z(beout))
    nc = _get_program(flags)

    wq_b = _bf(np.asarray(Wq, np.float32) * sc)
    wk_b, wv_b, wo_b, wso_b = _bf(Wk), _bf(Wv), _bf(Wo), _bf(Wso)
    wi_b, wout_b = _bf(Wi), _bf(Wout)
    identb = _bf(np.eye(128))
    onesr = _bf(np.ones((1, 512)))
    ones64 = _bf(np.ones((128, 64)))

    brow = np.zeros((7, FF), np.float32)
    brow[0, :E] = np.asarray(bq, np.float32) * sc
    for i, v in enumerate((bk, bv, bo, bso, bout)):
        brow[i + 1, :E] = v
    brow[6, :] = bi
    brow = _bf(brow)
    bicol = np.asarray(bi, np.float32).reshape(FF // 128, 128).T.copy()
    gbt = np.zeros((128, 4 * E + 512), np.float32)
    for i, g in enumerate((gso, gout, beso, beout)):   # g1|g2|b1|b2
        gbt[:, i * E:(i + 1) * E] = np.broadcast_to(
            np.asarray(g, np.float32).reshape(1, E), (128, E))

    in_maps = []
    for c in range(NCORES):
        xs = x[c * BL:(c + 1) * BL].reshape(T, E)
        xt = _bf(np.ascontiguousarray(xs.T))
        ms = np.asarray(mask[c * BL:(c + 1) * BL]).reshape(BL, S)
        # mbias[p, b*4 + jc] = 0/-87 for key token jc*128+p of batch b
        mbias = np.where(ms == 0, np.float32(MASK_NEG), np.float32(0.0))
        mbias = np.ascontiguousarray(
            mbias.reshape(BL, 4, 128).transpose(2, 0, 1).reshape(128, BL * 4)
        ).astype(np.float32)
        in_maps.append({
            "xt": xt, "wq": wq_b, "wk": wk_b, "wv": wv_b, "wo": wo_b,
            "wso": wso_b, "wi": wi_b, "wout": wout_b, "mbias": mbias,
            "ident": identb, "onesrow": onesr, "ones64": ones64,
            "brow": brow, "bicol": bicol, "gb": gbt,
        })

    trace = os.environ.get("KERNEL_TRACE", "0") == "1"
    res = run_bass_kernel_spmd(nc, in_maps, core_ids=list(range(NCORES)),
                               trace=trace)
    if trace and res.exec_time_ns is not None:
        print(f"HW exec time: {res.exec_time_ns} ns")
        if res.instructions_and_trace is not None:
            print(f"trace: {res.instructions_and_trace[1]}")
    out = np.concatenate([r["out"].reshape(BL, S, E) for r in res.results],
                         axis=0)
    return np.ascontiguousarray(out.astype(np.float32))
